# revision 5
# baseline (speedup 1.0000x reference)
"""BlackwellLinear Trainium2 kernel: 2:4 sparsity + int8 fake-quant + x @ w.T + bias.

Full inputs in, full output out. Hybrid sharding across 8 NeuronCores:
4 token groups x 2 out_feature groups. Each core computes
y[tg-block, fg-block] = x[tg] @ w[fg].T * scale + bias[fg], and runs the
module's weight prep (2:4 sparsify + int8 fake-quant) for its own out_feature
half -- halving the elementwise prep work per core vs pure data-parallel
(that prep chain is what gates the single-pass matmul pipeline start).
No collectives: the global absmax over the sparsified weight equals absmax of
|w| (the global max always survives top-2-of-4 selection), and each core
computes it from its own fp32 half plus a compact fp16 shadow of the other
half (scale perturbation ~2^-11, far inside the error budget).

Host does layout/encoding only: transposes, fp16 encodes of x and the shadow
half, and a phase-major permutation of the in_features axis
(p <-> 4*(p%256) + p//256) applied to both x.T and w.T. The permutation makes
each group-of-4 (the 2:4 unit) span four k-tiles at the SAME partition/column
coordinates, so sparsify+quantize is contiguous full-width elementwise ops and
the quantized weight lands directly in [in_f, out_f] (lhsT) layout. A
contraction-axis permutation applied to both operands leaves the matmul
result unchanged. All module math (threshold, mask, quantize, matmul, bias)
runs on device.

Engine budget (per-core): PE does one fp16 pass (512 MMs of N=512 -- the
dense roofline). DVE: threshold trees, masks, absmax reduces, PSUM evictions.
ACT: |w|, the two magic-round steps. Pool: mask-apply multiplies, allreduce.
All weight DMAs are issued before any compute op so the ACT stream cannot
stall a weight load; the other-half shadow and bias ride single DMAs.

Numerics (harness gate rel_err < 2e-2; this kernel lands ~1e-3):
  s   = absmax * (1/qmax)                  (1 ulp from fl(absmax/qmax))
  inv ~= 1/s                               (HW reciprocal + 1 Newton step)
  k   = rne(w * inv)                       (magic-constant RNE round on ACT)
  q   = k * mask                           (2:4 mask; integers are fp16-exact)
  y   = s * (x16 @ q.T) + bias             (scale folded into PSUM eviction)
x is fp16 (rel 2^-11); products are exact into fp32 PSUM. Threshold/mask
compares run on fp32 weights so near-tie selections match the reference.
"""

import numpy as np

N_CORES = 8
P = 128
IN_F = 1024
OUT_F = 1024
TOKENS = 32768
T_GROUPS = 4
F_GROUPS = 2
TOK_PC = TOKENS // T_GROUPS  # 8192 tokens per core
OUT_PC = OUT_F // F_GROUPS  # 512 out_features per core
K_TILES = IN_F // P  # 8
M_TILES = OUT_PC // P  # 4
TB_TOK = 1024  # token block per x strip
N_TB = TOK_PC // TB_TOK  # 8
MM_N = 512  # matmul moving free dim (one PSUM bank of fp32)
TJ = TB_TOK // MM_N  # 2

MAGIC = 12582912.0  # 1.5 * 2**23: (v + MAGIC) - MAGIC == RNE round, |v| <= 2**22

# k-tile order: range-0 tiles (phases of groups 0..127) first so the range-0
# threshold -> mask -> quant chain completes with half the weight DMA landed;
# PE accumulates k-tiles in this same order (sum order is commutative).
KT_ORDER = (0, 2, 4, 6, 1, 3, 5, 7)

# phase-major permutation of the in_features axis: position p holds original
# feature 4*(p%256) + p//256, so k-tile kt covers phase kt//2 of group range
# (kt%2)*128..+128 and the four phases of a group share partition/column coords
_PERM = (4 * (np.arange(IN_F) % 256) + np.arange(IN_F) // 256).astype(np.int64)

_CACHE = {}


def _build(qmax: float):
    from contextlib import ExitStack

    import concourse.tile as tile
    import concourse.mybir as mybir
    from concourse import bacc, bass_isa

    f32 = mybir.dt.float32
    f16 = mybir.dt.float16
    Alu = mybir.AluOpType
    Act = mybir.ActivationFunctionType

    inv_qmax = float(np.float32(1.0) / np.float32(qmax))

    nc = bacc.Bacc("TRN2", target_bir_lowering=False, debug=False)
    xth = nc.dram_tensor("xth", [IN_F, TOK_PC], f16, kind="ExternalInput").ap()
    # own out_f half of w.T (permuted in_f rows), fp32: exact 2:4 tie behavior
    wpo = nc.dram_tensor("wpo", [IN_F, OUT_PC], f32, kind="ExternalInput").ap()
    # other half, fp16 shadow packed [128, K_TILES*OUT_PC]: feeds absmax only
    wpx = nc.dram_tensor(
        "wpx", [P, K_TILES * OUT_PC], f16, kind="ExternalInput"
    ).ap()
    # bias packed [128, M_TILES]: column mi = bias[mi*128:(mi+1)*128]
    biasb = nc.dram_tensor("biasb", [P, M_TILES], f32, kind="ExternalInput").ap()
    yt = nc.dram_tensor("yt", [OUT_PC, TOK_PC], f16, kind="ExternalOutput").ap()

    with tile.TileContext(nc) as tc, ExitStack() as ctx:
        const = ctx.enter_context(tc.tile_pool(name="const", bufs=1))
        wnat_p = ctx.enter_context(tc.tile_pool(name="wnat", bufs=8))
        abs_p = ctx.enter_context(tc.tile_pool(name="absp", bufs=8))
        thr_p = ctx.enter_context(tc.tile_pool(name="thr", bufs=2))
        tt_p = ctx.enter_context(tc.tile_pool(name="ttmp", bufs=2))
        mask_p = ctx.enter_context(tc.tile_pool(name="mask", bufs=8))
        qtmp_p = ctx.enter_context(tc.tile_pool(name="qtmp", bufs=2))
        wqt_p = ctx.enter_context(tc.tile_pool(name="wqt", bufs=8))
        sc_p = ctx.enter_context(tc.tile_pool(name="sc", bufs=1))
        x_p = ctx.enter_context(tc.tile_pool(name="x", bufs=16))
        y_p = ctx.enter_context(tc.tile_pool(name="y", bufs=4))
        psum_mm = ctx.enter_context(tc.tile_pool(name="psmm", bufs=8, space="PSUM"))

        # ---- ALL weight-critical DMAs first: no compute op may precede a
        # weight dma_start in any engine stream ----
        wk = [None] * K_TILES
        for i, kt in enumerate(KT_ORDER):
            wt = wnat_p.tile([P, OUT_PC], f32, tag="wnat", name=f"wnat{kt}")
            (nc.sync if i % 2 == 0 else nc.scalar).dma_start(
                wt[:], wpo[kt * P : (kt + 1) * P, :]
            )
            wk[kt] = wt
        shx = const.tile([P, K_TILES * OUT_PC], f16, tag="shx")
        nc.scalar.dma_start(shx[:], wpx[:, :])
        biast = const.tile([P, M_TILES], f32, tag="biast")
        nc.scalar.dma_start(biast[:], biasb[:, :])

        # ---- |w| per own k-tile (ACT), arrival-pipelined ----
        ak = [None] * K_TILES
        for kt in KT_ORDER:
            a = abs_p.tile([P, OUT_PC], f32, tag="abs", name=f"abs{kt}")
            nc.scalar.activation(a[:], wk[kt][:], Act.Abs)
            ak[kt] = a

        def vts(out, in0, s1, op0, s2=None, op1=None):
            kw = {"op1": op1} if op1 is not None else {}
            nc.vector.tensor_scalar(
                out=out, in0=in0, scalar1=s1, scalar2=s2, op0=op0, **kw
            )

        def vtt(out, in0, in1, op):
            nc.vector.tensor_tensor(out=out, in0=in0, in1=in1, op=op)

        # ---- pre-scale DVE chain: tree partials that also yield the own-half
        # absmax, then the fp16 shadow max/min, then the global absmax ----
        tA, tB = {}, {}
        for r in (0, 1):
            a0, a1, a2, a3 = (ak[2 * j + r] for j in range(4))
            tA[r] = tt_p.tile([P, OUT_PC], f32, tag="tA", name=f"tA{r}")
            tB[r] = tt_p.tile([P, OUT_PC], f32, tag="tB", name=f"tB{r}")
            vtt(tA[r][:], a0[:], a1[:], Alu.max)
            vtt(tB[r][:], a2[:], a3[:], Alu.max)
        cm = sc_p.tile([P, 3], f32, tag="cm")
        for r in (0, 1):
            tmax = tt_p.tile([P, OUT_PC], f32, tag="tmax", name=f"tmax{r}")
            vtt(tmax[:], tA[r][:], tB[r][:], Alu.max)
            nc.vector.tensor_reduce(
                out=cm[:, r : r + 1], in_=tmax[:],
                axis=mybir.AxisListType.X, op=Alu.max,
            )
        nc.vector.tensor_reduce(
            out=cm[:, 2:3], in_=shx[:], axis=mybir.AxisListType.X, op=Alu.max
        )
        mn = sc_p.tile([P, 1], f32, tag="mn")
        nc.vector.tensor_reduce(
            out=mn[:], in_=shx[:], axis=mybir.AxisListType.X, op=Alu.min
        )
        nmn = sc_p.tile([P, 1], f32, tag="nmn")
        vts(nmn[:], mn[:], -1.0, Alu.mult)
        amc = sc_p.tile([P, 1], f32, tag="amc")
        nc.vector.reduce_max(amc[:], cm[:], axis=mybir.AxisListType.X)
        am0 = sc_p.tile([P, 1], f32, tag="am0")
        vtt(am0[:], amc[:], nmn[:], Alu.max)
        am = sc_p.tile([P, 1], f32, tag="am")
        nc.gpsimd.partition_all_reduce(
            am[:], am0[:], channels=P, reduce_op=bass_isa.ReduceOp.max
        )

        # ---- s = absmax/qmax (1 ulp); inv = 1/s (reciprocal + 1 Newton) ----
        s_t = sc_p.tile([P, 1], f32, tag="s")
        vts(s_t[:], am[:], inv_qmax, Alu.mult)
        r0 = sc_p.tile([P, 1], f32, tag="r0")
        nc.vector.reciprocal(r0[:], s_t[:])
        p1 = sc_p.tile([P, 1], f32, tag="p1")
        vtt(p1[:], s_t[:], r0[:], Alu.mult)
        e1 = sc_p.tile([P, 1], f32, tag="e1")
        vts(e1[:], p1[:], 2.0, Alu.subtract, -1.0, Alu.mult)  # 2 - s*r0
        inv_t = sc_p.tile([P, 1], f32, tag="inv")
        vtt(inv_t[:], r0[:], e1[:], Alu.mult)

        magic_t = sc_p.tile([P, 1], f32, tag="magic")
        nc.gpsimd.memset(magic_t[:], MAGIC)
        nmagic_t = sc_p.tile([P, 1], f32, tag="nmagic")
        nc.gpsimd.memset(nmagic_t[:], -MAGIC)

        # ---- 2:4 threshold + masks (DVE, fp32-exact ties); thr_r = 2nd
        # largest of each group = max(min of pair maxes, max of pair mins) ----
        thr = {}

        def build_thr(r):
            a0, a1, a2, a3 = (ak[2 * j + r] for j in range(4))
            t1 = tt_p.tile([P, OUT_PC], f32, tag="t1", name=f"t1_{r}")
            tB2 = tt_p.tile([P, OUT_PC], f32, tag="tB2", name=f"tB2_{r}")
            tC = tt_p.tile([P, OUT_PC], f32, tag="tC", name=f"tC_{r}")
            tr = thr_p.tile([P, OUT_PC], f32, tag="thr", name=f"thr_{r}")
            vtt(t1[:], tA[r][:], tB[r][:], Alu.min)
            vtt(tB2[:], a0[:], a1[:], Alu.min)
            vtt(tC[:], a2[:], a3[:], Alu.min)
            vtt(tB2[:], tB2[:], tC[:], Alu.max)
            vtt(tr[:], t1[:], tB2[:], Alu.max)
            thr[r] = tr

        masks = {}

        def build_mask(kt):
            m = mask_p.tile([P, OUT_PC], f16, tag="mask", name=f"m{kt}")
            vtt(m[:], ak[kt][:], thr[kt % 2][:], Alu.is_ge)
            masks[kt] = m

        build_thr(0)
        for kt in (0, 2, 4, 6):
            build_mask(kt)
        build_thr(1)
        for kt in (1, 3, 5, 7):
            build_mask(kt)

        # ---- quantize per k-tile: magic-round on ACT, mask-apply on Pool ----
        wqt_by_kt = {}
        for kt in KT_ORDER:
            q0 = qtmp_p.tile([P, OUT_PC], f32, tag="q0", name=f"q0_{kt}")
            nc.scalar.activation(
                q0[:], wk[kt][:], Act.Identity, bias=magic_t[:], scale=inv_t[:]
            )
            qr = qtmp_p.tile([P, OUT_PC], f16, tag="qr", name=f"qr_{kt}")
            nc.scalar.activation(qr[:], q0[:], Act.Identity, bias=nmagic_t[:])
            q16 = wqt_p.tile([P, OUT_PC], f16, tag="q16", name=f"q16_{kt}")
            nc.gpsimd.tensor_tensor(
                out=q16[:], in0=qr[:], in1=masks[kt][:], op=Alu.mult
            )
            wqt_by_kt[kt] = q16
        wqt = [wqt_by_kt[kt] for kt in range(K_TILES)]

        # ---- main matmul: yt[m, t] = sum_k wqt[k,m].T @ xh[k,t] ----
        # x loads on sync; evictions on DVE (tensor_scalar with per-partition
        # scale s and bias APs); y stores on scalar
        for tb in range(N_TB):
            xh = [None] * K_TILES
            for ki in KT_ORDER:
                sl_p = slice(ki * P, (ki + 1) * P)
                sl_t = slice(tb * TB_TOK, (tb + 1) * TB_TOK)
                xht = x_p.tile([P, TB_TOK], f16, tag="xh", name=f"xh{tb}_{ki}")
                nc.sync.dma_start(xht[:], xth[sl_p, sl_t])
                xh[ki] = xht

            def evict(mi, ps_tj):
                ysb = y_p.tile([P, TB_TOK], f16, tag="ysb", name=f"y{tb}_{mi}")
                for tj in range(TJ):
                    nc.vector.tensor_scalar(
                        out=ysb[:, tj * MM_N : (tj + 1) * MM_N],
                        in0=ps_tj[tj][:],
                        scalar1=s_t[:],
                        scalar2=biast[:, mi : mi + 1],
                        op0=Alu.mult,
                        op1=Alu.add,
                    )
                tcol = tb * TB_TOK
                nc.scalar.dma_start(
                    yt[mi * P : (mi + 1) * P, tcol : tcol + TB_TOK], ysb[:]
                )

            if tb == 0:
                # k-outer sweep over all 4 m-tiles (8 PSUM banks): PE starts
                # on the first quantized k-tile, consuming at the prep pace
                ps = {
                    (mi, tj): psum_mm.tile(
                        [P, MM_N], f32, tag="ps", name=f"ps0_{mi}_{tj}"
                    )
                    for mi in range(M_TILES)
                    for tj in range(TJ)
                }
                for kpos, ki in enumerate(KT_ORDER):
                    for mi in range(M_TILES):
                        lhsT = wqt[ki][:, mi * P : (mi + 1) * P]
                        for tj in range(TJ):
                            nc.tensor.matmul(
                                ps[mi, tj][:],
                                lhsT,
                                xh[ki][:, tj * MM_N : (tj + 1) * MM_N],
                                start=(kpos == 0),
                                stop=(kpos == K_TILES - 1),
                            )
                for mi in range(M_TILES):
                    evict(mi, [ps[mi, tj] for tj in range(TJ)])
            else:
                for mi in range(M_TILES):
                    ps = [
                        psum_mm.tile(
                            [P, MM_N], f32, tag="ps", name=f"ps{tb}_{mi}_{tj}"
                        )
                        for tj in range(TJ)
                    ]
                    for kpos, ki in enumerate(KT_ORDER):
                        lhsT = wqt[ki][:, mi * P : (mi + 1) * P]
                        for tj in range(TJ):
                            nc.tensor.matmul(
                                ps[tj][:],
                                lhsT,
                                xh[ki][:, tj * MM_N : (tj + 1) * MM_N],
                                start=(kpos == 0),
                                stop=(kpos == K_TILES - 1),
                            )
                    evict(mi, ps)

    nc.compile()
    return nc


def _get(qmax: float):
    key = qmax
    if key not in _CACHE:
        _CACHE[key] = _build(qmax)
    return _CACHE[key]


def host_prep(x, weight):
    """Host-side input re-encoding: transpose, phase-major permute the in_f
    axis, fp16 encodes, and pack the shadow/bias layouts. Pure layout."""
    xt = np.ascontiguousarray(x.T)[_PERM]  # [IN_F perm, TOKENS]
    xth = xt.astype(np.float16)
    wp = np.ascontiguousarray(weight.T[_PERM])  # [IN_F perm, OUT_F] fp32
    wp16 = wp.astype(np.float16)
    return xth, wp, wp16


LAST_EXEC_NS = None


def kernel(x, weight, bias, precision, _trace_dir=None):
    global LAST_EXEC_NS
    from concourse.bass_utils import run_bass_kernel_spmd

    x = np.asarray(x, dtype=np.float32)
    weight = np.asarray(weight, dtype=np.float32)
    bias = np.asarray(bias, dtype=np.float32)
    prec = int(np.asarray(precision))
    qmax = float(2 ** (prec - 1) - 1)

    nc = _get(qmax)

    xth, wp, wp16 = host_prep(x, weight)
    in_maps = []
    for c in range(N_CORES):
        tg, fg = c // F_GROUPS, c % F_GROUPS
        o0, o1 = fg * OUT_PC, (fg + 1) * OUT_PC
        x0, x1 = (1 - fg) * OUT_PC, (2 - fg) * OUT_PC
        shadow = wp16[:, x0:x1]  # [1024, 512] fp16, other half
        shadow_packed = np.ascontiguousarray(
            shadow.reshape(K_TILES, P, OUT_PC).transpose(1, 0, 2).reshape(
                P, K_TILES * OUT_PC
            )
        )
        bias_half = bias[o0:o1]
        bias_packed = np.ascontiguousarray(
            bias_half.reshape(M_TILES, P).T
        )  # [128, M_TILES]
        in_maps.append(
            {
                "xth": np.ascontiguousarray(
                    xth[:, tg * TOK_PC : (tg + 1) * TOK_PC]
                ),
                "wpo": np.ascontiguousarray(wp[:, o0:o1]),
                "wpx": shadow_packed,
                "biasb": bias_packed,
            }
        )
    kw = {}
    if _trace_dir is not None:
        kw = {"trace": True, "tmpdir": _trace_dir}
    res = run_bass_kernel_spmd(nc, in_maps, list(range(N_CORES)), **kw)
    LAST_EXEC_NS = res.exec_time_ns
    y = np.empty((TOKENS, OUT_F), dtype=np.float32)
    for c in range(N_CORES):
        tg, fg = c // F_GROUPS, c % F_GROUPS
        y[tg * TOK_PC : (tg + 1) * TOK_PC, fg * OUT_PC : (fg + 1) * OUT_PC] = (
            res.results[c]["yt"].T.astype(np.float32)
        )
    return y


# revision 8
# speedup vs baseline: 1.0311x; 1.0311x over previous
"""BlackwellLinear Trainium2 kernel: 2:4 sparsity + int8 fake-quant + x @ w.T + bias.

Full inputs in, full output out. Hybrid sharding across 8 NeuronCores:
4 token groups x 2 out_feature groups. Each core computes
y[tg-block, fg-block] = x[tg] @ w[fg].T * scale + bias[fg], and runs the
module's weight prep (2:4 sparsify + int8 fake-quant) for its own out_feature
half -- halving the elementwise prep work per core vs pure data-parallel
(that prep chain is what gates the single-pass matmul pipeline start).
No collectives: the global absmax over the sparsified weight equals absmax of
|w| (the global max always survives top-2-of-4 selection), and each core
computes it from its own fp32 half plus a compact fp16 shadow of the other
half (scale perturbation ~2^-11, far inside the error budget).

Host does layout/encoding only: transposes, fp16 encodes of x and the shadow
half, and a phase-major permutation of the in_features axis
(p <-> 4*(p%256) + p//256) applied to both x.T and w.T. The permutation makes
each group-of-4 (the 2:4 unit) span four k-tiles at the SAME partition/column
coordinates, so sparsify+quantize is contiguous full-width elementwise ops and
the quantized weight lands directly in [in_f, out_f] (lhsT) layout. A
contraction-axis permutation applied to both operands leaves the matmul
result unchanged. All module math (threshold, mask, quantize, matmul, bias)
runs on device.

Engine budget (per-core): PE does one fp16 pass (512 MMs of N=512 -- the
dense roofline). DVE: threshold trees, masks, absmax reduces, PSUM evictions.
ACT: |w|, the two magic-round steps. Pool: mask-apply multiplies, allreduce.
All weight DMAs are issued before any compute op so the ACT stream cannot
stall a weight load; the other-half shadow and bias ride single DMAs.

Numerics (harness gate rel_err < 2e-2; this kernel lands ~1e-3):
  s   = absmax * (1/qmax)                  (1 ulp from fl(absmax/qmax))
  inv ~= 1/s                               (HW reciprocal + 1 Newton step)
  k   = rne(w * inv)                       (magic-constant RNE round on ACT)
  q   = k * mask                           (2:4 mask; integers are fp16-exact)
  y   = s * (x16 @ q.T) + bias             (scale folded into PSUM eviction)
x is fp16 (rel 2^-11); products are exact into fp32 PSUM. Threshold/mask
compares run on fp32 weights so near-tie selections match the reference.
"""

import numpy as np

N_CORES = 8
P = 128
IN_F = 1024
OUT_F = 1024
TOKENS = 32768
T_GROUPS = 4
F_GROUPS = 2
TOK_PC = TOKENS // T_GROUPS  # 8192 tokens per core
OUT_PC = OUT_F // F_GROUPS  # 512 out_features per core
K_TILES = IN_F // P  # 8
M_TILES = OUT_PC // P  # 4
TB_TOK = 1024  # token block per x strip
N_TB = TOK_PC // TB_TOK  # 8
MM_N = 512  # matmul moving free dim (one PSUM bank of fp32)
TJ = TB_TOK // MM_N  # 2

MAGIC = 12582912.0  # 1.5 * 2**23: (v + MAGIC) - MAGIC == RNE round, |v| <= 2**22

# k-tile order: range-0 tiles (phases of groups 0..127) first so the range-0
# threshold -> mask -> quant chain completes with half the weight DMA landed;
# PE accumulates k-tiles in this same order (sum order is commutative).
KT_ORDER = (0, 2, 4, 6, 1, 3, 5, 7)

# phase-major permutation of the in_features axis: position p holds original
# feature 4*(p%256) + p//256, so k-tile kt covers phase kt//2 of group range
# (kt%2)*128..+128 and the four phases of a group share partition/column coords
_PERM = (4 * (np.arange(IN_F) % 256) + np.arange(IN_F) // 256).astype(np.int64)

_CACHE = {}


def _build(qmax: float):
    from contextlib import ExitStack

    import concourse.tile as tile
    import concourse.mybir as mybir
    from concourse import bacc, bass_isa

    f32 = mybir.dt.float32
    f16 = mybir.dt.float16
    Alu = mybir.AluOpType
    Act = mybir.ActivationFunctionType

    inv_qmax = float(np.float32(1.0) / np.float32(qmax))

    nc = bacc.Bacc("TRN2", target_bir_lowering=False, debug=False)
    xth = nc.dram_tensor("xth", [IN_F, TOK_PC], f16, kind="ExternalInput").ap()
    # own out_f half of w.T (permuted in_f rows), fp32: exact 2:4 tie behavior
    wpo = nc.dram_tensor("wpo", [IN_F, OUT_PC], f32, kind="ExternalInput").ap()
    # other half, fp16 shadow packed [128, K_TILES*OUT_PC]: feeds absmax only
    wpx = nc.dram_tensor(
        "wpx", [P, K_TILES * OUT_PC], f16, kind="ExternalInput"
    ).ap()
    # bias packed [128, M_TILES]: column mi = bias[mi*128:(mi+1)*128]
    biasb = nc.dram_tensor("biasb", [P, M_TILES], f32, kind="ExternalInput").ap()
    yt = nc.dram_tensor("yt", [OUT_PC, TOK_PC], f16, kind="ExternalOutput").ap()

    with tile.TileContext(nc) as tc, ExitStack() as ctx:
        const = ctx.enter_context(tc.tile_pool(name="const", bufs=1))
        wnat_p = ctx.enter_context(tc.tile_pool(name="wnat", bufs=8))
        abs_p = ctx.enter_context(tc.tile_pool(name="absp", bufs=8))
        thr_p = ctx.enter_context(tc.tile_pool(name="thr", bufs=2))
        tt_p = ctx.enter_context(tc.tile_pool(name="ttmp", bufs=2))
        mask_p = ctx.enter_context(tc.tile_pool(name="mask", bufs=8))
        qtmp_p = ctx.enter_context(tc.tile_pool(name="qtmp", bufs=2))
        wqt_p = ctx.enter_context(tc.tile_pool(name="wqt", bufs=8))
        sc_p = ctx.enter_context(tc.tile_pool(name="sc", bufs=1))
        x_p = ctx.enter_context(tc.tile_pool(name="x", bufs=16))
        y_p = ctx.enter_context(tc.tile_pool(name="y", bufs=4))
        psum_mm = ctx.enter_context(tc.tile_pool(name="psmm", bufs=8, space="PSUM"))

        # ---- ALL weight-critical DMAs first: no compute op may precede a
        # weight dma_start in any engine stream. Queue FIFO ordering keeps the
        # x strips (emitted later on sync) behind the weights on the wire. ----
        wk = [None] * K_TILES
        for i, kt in enumerate(KT_ORDER):
            wt = wnat_p.tile([P, OUT_PC], f32, tag="wnat", name=f"wnat{kt}")
            (nc.sync if i % 2 == 0 else nc.scalar).dma_start(
                wt[:], wpo[kt * P : (kt + 1) * P, :]
            )
            wk[kt] = wt
        SHH = K_TILES * OUT_PC // 2
        shx = const.tile([P, K_TILES * OUT_PC], f16, tag="shx")
        nc.sync.dma_start(shx[:, 0:SHH], wpx[:, 0:SHH])
        nc.sync.dma_start(shx[:, SHH:], wpx[:, SHH:])
        biast = const.tile([P, M_TILES], f32, tag="biast")
        nc.scalar.dma_start(biast[:], biasb[:, :])

        # ---- |w| per own k-tile (ACT, arrival-pipelined), then |shadow| ----
        ak = [None] * K_TILES
        for kt in KT_ORDER:
            a = abs_p.tile([P, OUT_PC], f32, tag="abs", name=f"abs{kt}")
            nc.scalar.activation(a[:], wk[kt][:], Act.Abs)
            ak[kt] = a
        ash = const.tile([P, K_TILES * OUT_PC], f16, tag="ash")
        nc.scalar.activation(ash[:, 0:SHH], shx[:, 0:SHH], Act.Abs)
        nc.scalar.activation(ash[:, SHH:], shx[:, SHH:], Act.Abs)

        def vts(out, in0, s1, op0, s2=None, op1=None):
            kw = {"op1": op1} if op1 is not None else {}
            nc.vector.tensor_scalar(
                out=out, in0=in0, scalar1=s1, scalar2=s2, op0=op0, **kw
            )

        def vtt(out, in0, in1, op):
            nc.vector.tensor_tensor(out=out, in0=in0, in1=in1, op=op)

        # ---- DVE chain, ordered for earliest global-absmax + first mask:
        # own-half tree partials double as the own absmax; the fp16 shadow
        # reduces run after the range-0 threshold/mask so the PE gate
        # (inv -> q16[kt0]) resolves as early as possible ----
        tA, tB = {}, {}
        for r in (0, 1):
            a0, a1, a2, a3 = (ak[2 * j + r] for j in range(4))
            tA[r] = tt_p.tile([P, OUT_PC], f32, tag="tA", name=f"tA{r}")
            tB[r] = tt_p.tile([P, OUT_PC], f32, tag="tB", name=f"tB{r}")
            vtt(tA[r][:], a0[:], a1[:], Alu.max)
            vtt(tB[r][:], a2[:], a3[:], Alu.max)
        cm = sc_p.tile([P, 4], f32, tag="cm")
        for r in (0, 1):
            tmax = tt_p.tile([P, OUT_PC], f32, tag="tmax", name=f"tmax{r}")
            vtt(tmax[:], tA[r][:], tB[r][:], Alu.max)
            nc.vector.tensor_reduce(
                out=cm[:, r : r + 1], in_=tmax[:],
                axis=mybir.AxisListType.X, op=Alu.max,
            )

        # range-0 threshold + first mask before the shadow reduces
        thr = {}
        masks = {}

        def build_thr(r):
            a0, a1, a2, a3 = (ak[2 * j + r] for j in range(4))
            t1 = tt_p.tile([P, OUT_PC], f32, tag="t1", name=f"t1_{r}")
            tB2 = tt_p.tile([P, OUT_PC], f32, tag="tB2", name=f"tB2_{r}")
            tC = tt_p.tile([P, OUT_PC], f32, tag="tC", name=f"tC_{r}")
            tr = thr_p.tile([P, OUT_PC], f32, tag="thr", name=f"thr_{r}")
            vtt(t1[:], tA[r][:], tB[r][:], Alu.min)
            vtt(tB2[:], a0[:], a1[:], Alu.min)
            vtt(tC[:], a2[:], a3[:], Alu.min)
            vtt(tB2[:], tB2[:], tC[:], Alu.max)
            vtt(tr[:], t1[:], tB2[:], Alu.max)
            thr[r] = tr

        def build_mask(kt):
            m = mask_p.tile([P, OUT_PC], f16, tag="mask", name=f"m{kt}")
            vtt(m[:], ak[kt][:], thr[kt % 2][:], Alu.is_ge)
            masks[kt] = m

        build_thr(0)
        build_mask(0)

        # shadow absmax (fp16 reduce runs at 1x; split for pipelining)
        nc.vector.tensor_reduce(
            out=cm[:, 2:3], in_=ash[:, 0:SHH], axis=mybir.AxisListType.X,
            op=Alu.max,
        )
        nc.vector.tensor_reduce(
            out=cm[:, 3:4], in_=ash[:, SHH:], axis=mybir.AxisListType.X,
            op=Alu.max,
        )
        amc = sc_p.tile([P, 1], f32, tag="amc")
        nc.vector.reduce_max(amc[:], cm[:], axis=mybir.AxisListType.X)
        am = sc_p.tile([P, 1], f32, tag="am")
        nc.gpsimd.partition_all_reduce(
            am[:], amc[:], channels=P, reduce_op=bass_isa.ReduceOp.max
        )

        # ---- s = absmax/qmax (1 ulp); inv = 1/s (reciprocal + 1 Newton) ----
        s_t = sc_p.tile([P, 1], f32, tag="s")
        vts(s_t[:], am[:], inv_qmax, Alu.mult)
        r0 = sc_p.tile([P, 1], f32, tag="r0")
        nc.vector.reciprocal(r0[:], s_t[:])
        p1 = sc_p.tile([P, 1], f32, tag="p1")
        vtt(p1[:], s_t[:], r0[:], Alu.mult)
        e1 = sc_p.tile([P, 1], f32, tag="e1")
        vts(e1[:], p1[:], 2.0, Alu.subtract, -1.0, Alu.mult)  # 2 - s*r0
        inv_t = sc_p.tile([P, 1], f32, tag="inv")
        vtt(inv_t[:], r0[:], e1[:], Alu.mult)

        magic_t = sc_p.tile([P, 1], f32, tag="magic")
        nc.gpsimd.memset(magic_t[:], MAGIC)
        nmagic_t = sc_p.tile([P, 1], f32, tag="nmagic")
        nc.gpsimd.memset(nmagic_t[:], -MAGIC)

        # ---- remaining masks: range-0 rest, then range-1 tree + masks ----
        for kt in (2, 4, 6):
            build_mask(kt)
        build_thr(1)
        for kt in (1, 3, 5, 7):
            build_mask(kt)

        # ---- quantize per k-tile: magic-round on ACT, mask-apply on Pool ----
        wqt_by_kt = {}
        for kt in KT_ORDER:
            q0 = qtmp_p.tile([P, OUT_PC], f32, tag="q0", name=f"q0_{kt}")
            nc.scalar.activation(
                q0[:], wk[kt][:], Act.Identity, bias=magic_t[:], scale=inv_t[:]
            )
            qr = qtmp_p.tile([P, OUT_PC], f16, tag="qr", name=f"qr_{kt}")
            nc.scalar.activation(qr[:], q0[:], Act.Identity, bias=nmagic_t[:])
            q16 = wqt_p.tile([P, OUT_PC], f16, tag="q16", name=f"q16_{kt}")
            nc.gpsimd.tensor_tensor(
                out=q16[:], in0=qr[:], in1=masks[kt][:], op=Alu.mult
            )
            wqt_by_kt[kt] = q16
        wqt = [wqt_by_kt[kt] for kt in range(K_TILES)]

        # ---- main matmul: yt[m, t] = sum_k wqt[k,m].T @ xh[k,t] ----
        # x loads on sync; evictions on DVE (tensor_scalar with per-partition
        # scale s and bias APs); y stores on scalar
        for tb in range(N_TB):
            xh = [None] * K_TILES
            for ki in KT_ORDER:
                sl_p = slice(ki * P, (ki + 1) * P)
                sl_t = slice(tb * TB_TOK, (tb + 1) * TB_TOK)
                xht = x_p.tile([P, TB_TOK], f16, tag="xh", name=f"xh{tb}_{ki}")
                nc.sync.dma_start(xht[:], xth[sl_p, sl_t])
                xh[ki] = xht

            def evict(mi, ps_tj):
                ysb = y_p.tile([P, TB_TOK], f16, tag="ysb", name=f"y{tb}_{mi}")
                for tj in range(TJ):
                    nc.vector.tensor_scalar(
                        out=ysb[:, tj * MM_N : (tj + 1) * MM_N],
                        in0=ps_tj[tj][:],
                        scalar1=s_t[:],
                        scalar2=biast[:, mi : mi + 1],
                        op0=Alu.mult,
                        op1=Alu.add,
                    )
                tcol = tb * TB_TOK
                nc.scalar.dma_start(
                    yt[mi * P : (mi + 1) * P, tcol : tcol + TB_TOK], ysb[:]
                )

            if tb == 0:
                # k-outer sweep over all 4 m-tiles (8 PSUM banks): PE starts
                # on the first quantized k-tile, consuming at the prep pace
                ps = {
                    (mi, tj): psum_mm.tile(
                        [P, MM_N], f32, tag="ps", name=f"ps0_{mi}_{tj}"
                    )
                    for mi in range(M_TILES)
                    for tj in range(TJ)
                }
                for kpos, ki in enumerate(KT_ORDER):
                    for mi in range(M_TILES):
                        lhsT = wqt[ki][:, mi * P : (mi + 1) * P]
                        for tj in range(TJ):
                            nc.tensor.matmul(
                                ps[mi, tj][:],
                                lhsT,
                                xh[ki][:, tj * MM_N : (tj + 1) * MM_N],
                                start=(kpos == 0),
                                stop=(kpos == K_TILES - 1),
                            )
                for mi in range(M_TILES):
                    evict(mi, [ps[mi, tj] for tj in range(TJ)])
            else:
                for mi in range(M_TILES):
                    ps = [
                        psum_mm.tile(
                            [P, MM_N], f32, tag="ps", name=f"ps{tb}_{mi}_{tj}"
                        )
                        for tj in range(TJ)
                    ]
                    for kpos, ki in enumerate(KT_ORDER):
                        lhsT = wqt[ki][:, mi * P : (mi + 1) * P]
                        for tj in range(TJ):
                            nc.tensor.matmul(
                                ps[tj][:],
                                lhsT,
                                xh[ki][:, tj * MM_N : (tj + 1) * MM_N],
                                start=(kpos == 0),
                                stop=(kpos == K_TILES - 1),
                            )
                    evict(mi, ps)

    nc.compile()
    return nc


def _get(qmax: float):
    key = qmax
    if key not in _CACHE:
        _CACHE[key] = _build(qmax)
    return _CACHE[key]


def host_prep(x, weight):
    """Host-side input re-encoding: transpose, phase-major permute the in_f
    axis, fp16 encodes, and pack the shadow/bias layouts. Pure layout."""
    xt = np.ascontiguousarray(x.T)[_PERM]  # [IN_F perm, TOKENS]
    xth = xt.astype(np.float16)
    wp = np.ascontiguousarray(weight.T[_PERM])  # [IN_F perm, OUT_F] fp32
    wp16 = wp.astype(np.float16)
    return xth, wp, wp16


LAST_EXEC_NS = None


def kernel(x, weight, bias, precision, _trace_dir=None):
    global LAST_EXEC_NS
    from concourse.bass_utils import run_bass_kernel_spmd

    x = np.asarray(x, dtype=np.float32)
    weight = np.asarray(weight, dtype=np.float32)
    bias = np.asarray(bias, dtype=np.float32)
    prec = int(np.asarray(precision))
    qmax = float(2 ** (prec - 1) - 1)

    nc = _get(qmax)

    xth, wp, wp16 = host_prep(x, weight)
    in_maps = []
    for c in range(N_CORES):
        tg, fg = c // F_GROUPS, c % F_GROUPS
        o0, o1 = fg * OUT_PC, (fg + 1) * OUT_PC
        x0, x1 = (1 - fg) * OUT_PC, (2 - fg) * OUT_PC
        shadow = wp16[:, x0:x1]  # [1024, 512] fp16, other half
        shadow_packed = np.ascontiguousarray(
            shadow.reshape(K_TILES, P, OUT_PC).transpose(1, 0, 2).reshape(
                P, K_TILES * OUT_PC
            )
        )
        bias_half = bias[o0:o1]
        bias_packed = np.ascontiguousarray(
            bias_half.reshape(M_TILES, P).T
        )  # [128, M_TILES]
        in_maps.append(
            {
                "xth": np.ascontiguousarray(
                    xth[:, tg * TOK_PC : (tg + 1) * TOK_PC]
                ),
                "wpo": np.ascontiguousarray(wp[:, o0:o1]),
                "wpx": shadow_packed,
                "biasb": bias_packed,
            }
        )
    kw = {}
    if _trace_dir is not None:
        kw = {"trace": True, "tmpdir": _trace_dir}
    res = run_bass_kernel_spmd(nc, in_maps, list(range(N_CORES)), **kw)
    LAST_EXEC_NS = res.exec_time_ns
    y = np.empty((TOKENS, OUT_F), dtype=np.float32)
    for c in range(N_CORES):
        tg, fg = c // F_GROUPS, c % F_GROUPS
        y[tg * TOK_PC : (tg + 1) * TOK_PC, fg * OUT_PC : (fg + 1) * OUT_PC] = (
            res.results[c]["yt"].T.astype(np.float32)
        )
    return y
